# revision 27
# baseline (speedup 1.0000x reference)
"""Multi-head attention (B=2, S=2048, H=1024, 16 heads) on 8 TRN2 NeuronCores.

Sharding: tensor-parallel over heads x data-parallel over batch.
core = b * 4 + g handles batch b and head-group g (4 heads, 256 channels).

Device-side dataflow (bf16 operands, fp32 PSUM accumulation):
  - Everything stays in "transposed space" so every matmul contracts over the
    partition dim with no on-device transposes:
      x_t    [H, S]      = hidden[b].T                      (host-transposed)
      qk_T   [512, S]    = (Wqk_g x_t)                      rows: q(4 heads), k(4 heads)
      v      [S, 256]    = x w_v.T  (natural layout; lhsT = x_t chunks)
      st     [128k, q]   = k_T_h^T-contracted scores (transposed scores)
      pt     = exp(st * scale + mask[k])                    (ACT, bias = per-partition mask)
      av     [128, q]    = v_aug^T pt ; rows 0:64 = unnormalized out.T,
                           rows 64:128 = Z[q] replicated (v_aug cols 64:128 == 1)
      attn_T [256, S]    = av[:64] * reciprocal(av[64:128])
      out_t  [H, S]      = Wo_g^T-contracted partial output (transposed)
  - Host sums the 4 group partials per batch, transposes back, and adds the
    exact bias corrections: b_out plus w_out @ b_v (the ones-augmented-V
    identity makes the v-bias a constant channel offset).
"""

import numpy as np

import concourse.tile as tile
from concourse import bacc, mybir
from concourse.bass_utils import run_bass_kernel_spmd

B, S, H = 2, 2048, 1024
NH, HD = 16, 64
NCORES = 8
NGROUP = 4              # head groups = cores per batch
HPG = NH // NGROUP      # 4 heads per group
DG = HPG * HD           # 256 channels per group
P = 128
SCALE = float(HD) ** -0.5

FP32 = mybir.dt.float32
FP32R = mybir.dt.float32r
BF16 = mybir.dt.bfloat16

S_TILES = S // P        # 16 key/token tiles
HC = H // P             # 8 contraction chunks over H
QKR = 2 * DG            # 512 q+k rows
QKC = QKR // P          # 4 chunks of qk rows
TQ = 512                # token quarter for qkv streaming
NQT = S // TQ           # 4
QT = 1024               # q tile width in attention / out-proj
NQ = S // QT            # 2

_NC_CACHE = None
LAST_RESULT = None      # BassKernelResults of the most recent run (for test.py)


def _body(tc, x_t, wqk_t, wv_t, wo_t, bqk, mask, out_t):
    nc = tc.nc
    with (
        tc.tile_pool(name="const", bufs=1) as const,
        tc.tile_pool(name="big", bufs=1) as big,
        tc.tile_pool(name="pt_pool", bufs=8) as pt_pool,
        tc.tile_pool(name="rz_pool", bufs=2) as rz_pool,
        tc.tile_pool(name="osb_pool", bufs=3) as osb_pool,
        tc.tile_pool(name="ps", bufs=2, space="PSUM") as ps,
        tc.tile_pool(name="avps", bufs=2, space="PSUM") as avps,
        tc.tile_pool(name="iops", bufs=2, space="PSUM") as iops,
    ):
        # ---------- input DMAs, ordered by when compute needs them ----------
        # x chunks first (gates everything), split over three issuing engines
        # so descriptor generation (~0.6us per dma_start per sequencer)
        # overlaps; w_out / mask / biases land last (needed latest).
        x_sb = big.tile([P, HC, S], BF16, name="x_sb")
        x_r = x_t.rearrange("(c p) s -> p c s", p=P)
        dma_engines = (nc.sync, nc.scalar, nc.gpsimd)
        for hc in range(HC):
            dma_engines[hc % 3].dma_start(x_sb[:, hc, :], x_r[:, hc, :])

        wqk_sb = const.tile([P, HC, QKR], BF16, name="wqk_sb")
        nc.sync.dma_start(wqk_sb[:], wqk_t.rearrange("(c p) r -> p c r", p=P))
        wv_sb = const.tile([P, HC, DG], BF16, name="wv_sb")
        nc.scalar.dma_start(wv_sb[:], wv_t.rearrange("(c p) r -> p c r", p=P))
        bqk_sb = const.tile([P, QKC], FP32, name="bqk_sb")
        nc.scalar.dma_start(bqk_sb[:], bqk.rearrange("(c p) -> p c", p=P))
        mask_sb = const.tile([P, S_TILES], FP32, name="mask_sb")
        nc.scalar.dma_start(mask_sb[:], mask.rearrange("(c p) -> p c", p=P))
        wo_sb = const.tile([P, DG // P, H], BF16, name="wo_sb")
        nc.gpsimd.dma_start(wo_sb[:], wo_t.rearrange("(c p) r -> p c r", p=P))

        qk_sb = big.tile([P, QKC, S], BF16, name="qk_sb")
        # v_aug: per token tile / head: [v (64 cols) | ones (64 cols)]
        v_sb = big.tile([P, S_TILES, HPG, 2 * HD], BF16, name="v_sb")
        attn_sb = big.tile([P, DG // P, S], BF16, name="attn_sb")

        # ones half of v_aug: memset a bf16 staging tile, copy per token tile
        ones_sb = const.tile([P, HPG, HD], BF16, name="ones_sb")
        nc.vector.memset(ones_sb[:], 1.0)
        for tt in range(S_TILES):
            nc.vector.tensor_copy(v_sb[:, tt, :, HD:2 * HD], ones_sb[:])

        # ---------- qkv projection ----------
        # One [128,512] psum slot per group, double-buffered, v and qk groups
        # interleaved for PE density.  qk chunk order (0, 2, 1, 3): q+k of
        # head pair 0 first so attention starts while pair 1 still projects.
        def v_group(tp):
            # two token tiles (2*tp, 2*tp+1) side by side in one psum slot
            v_ps = iops.tile([P, 512], FP32, name="v_ps", tag="io")
            for half in range(2):
                tt = 2 * tp + half
                for hc in range(HC):
                    nc.tensor.matmul(
                        v_ps[:, half * DG:(half + 1) * DG],
                        lhsT=x_sb[:, hc, tt * P:(tt + 1) * P],
                        rhs=wv_sb[:, hc, :],
                        start=(hc == 0),
                        stop=(hc == HC - 1),
                    )
            nc.vector.tensor_copy(
                v_sb[:, 2 * tp:2 * tp + 2, :, 0:HD],
                v_ps[:].rearrange("p (t h d) -> p t h d", t=2, d=HD),
            )

        def qk_group(rc, i):
            qk_ps = iops.tile([P, 512], FP32, name="qk_ps", tag="io")
            for hc in range(HC):
                nc.tensor.matmul(
                    qk_ps[:],
                    lhsT=wqk_sb[:, hc, rc * P:(rc + 1) * P],
                    rhs=x_sb[:, hc, i * 512:(i + 1) * 512],
                    start=(hc == 0),
                    stop=(hc == HC - 1),
                )
            nc.vector.tensor_scalar_add(
                qk_sb[:, rc, i * 512:(i + 1) * 512],
                qk_ps[:],
                bqk_sb[:, rc:rc + 1],
            )

        # Window-0 dependencies first: q cols 0-511, then k/v in token order so
        # attention window 0 starts consuming k-tile kt / v-tile tt as they
        # appear; remaining q windows follow.
        qk_group(0, 0)
        for j in range(4):
            qk_group(2, j)
            v_group(2 * j)
            v_group(2 * j + 1)
            if j > 0:
                qk_group(0, j)

        # ---------- attention: head PAIRS packed into PE row-groups ----------
        # Heads (2*qc, 2*qc+1) live at partition offsets 0/64 of qk chunk qc,
        # so their score matmuls land in disjoint row groups (tile_position
        # (0,0) / (64,0)) and execute concurrently.  Their 512-wide score
        # tiles sit side by side in one [128,1024] PSUM tile so a single
        # N=1024 exp covers both (mask bias depends only on the k-partition).
        o_r = out_t.rearrange("(c p) s -> p c s", p=P)

        def out_proj_window(q5):
            """Output projection for one 512-wide q window (both attn chunks
            must be complete there).  Two j-chunks packed per psum slot."""
            qlo = q5 * 512
            for j in range(H // P):
                o_ps = iops.tile([P, 512], FP32, name="o_ps", tag="io")
                for kc in range(DG // P):
                    nc.tensor.matmul(
                        o_ps[:],
                        lhsT=wo_sb[:, kc, j * P:(j + 1) * P],
                        rhs=attn_sb[:, kc, qlo:qlo + 512],
                        start=(kc == 0),
                        stop=(kc == DG // P - 1),
                    )
                o_sb = osb_pool.tile([P, 512], BF16, name="o_sb", tag="osb")
                nc.vector.tensor_copy(o_sb[:], o_ps[:])
                nc.sync.dma_start(o_r[:, j, qlo:qlo + 512], o_sb[:])

        for qc in range(HPG // 2):
            for q5 in range(S // 512):
                qlo = q5 * 512
                av0 = avps.tile([P, 512], FP32, name="av0", tag="av")
                av1 = avps.tile([P, 512], FP32, name="av1", tag="av")
                for kt in range(S_TILES):
                    st = ps.tile([P, QT], FP32, name="st", tag="mm")
                    for half in range(2):
                        off = half * HD
                        nc.tensor.matmul(
                            st[:, half * 512:(half + 1) * 512],
                            lhsT=qk_sb[off:off + HD, 2 + qc,
                                       kt * P:(kt + 1) * P],
                            rhs=qk_sb[off:off + HD, qc, qlo:qlo + 512],
                            start=True,
                            stop=True,
                        )
                    pt = pt_pool.tile([P, QT], BF16, name="pt", tag="pt")
                    nc.scalar.activation(
                        pt[:], st[:],
                        mybir.ActivationFunctionType.Exp,
                        bias=mask_sb[:, kt:kt + 1],
                        scale=SCALE,
                    )
                    for half, av in ((0, av0), (1, av1)):
                        nc.tensor.matmul(
                            av[:],
                            lhsT=v_sb[:, kt, 2 * qc + half, :],
                            rhs=pt[:, half * 512:(half + 1) * 512],
                            start=(kt == 0),
                            stop=(kt == S_TILES - 1),
                        )
                for half, av in ((0, av0), (1, av1)):
                    off = half * HD
                    zc = rz_pool.tile([HD, 512], FP32, name="zc", tag="zc")
                    nc.vector.tensor_copy(zc[:], av[HD:2 * HD, :])
                    rz = rz_pool.tile([HD, 512], FP32, name="rz", tag="rz")
                    nc.vector.reciprocal_approx_fast(rz[:], zc[:])
                    nc.vector.tensor_mul(
                        attn_sb[off:off + HD, qc, qlo:qlo + 512],
                        av[0:HD, :],
                        rz[:],
                    )
                if qc == 0:
                    # pair 1's projection, emitted AFTER this window's
                    # attention ops: lower scheduler priority, so it fills PE
                    # gaps while ACT stays on the exp critical path
                    qk_group(1, q5)
                    qk_group(3, q5)
                else:
                    out_proj_window(q5)


def _build():
    nc = bacc.Bacc(
        "TRN2",
        target_bir_lowering=False,
        debug=False,
        enable_asserts=True,
        num_devices=NCORES,
    )
    x_t = nc.dram_tensor("x_t", [H, S], BF16, kind="ExternalInput").ap()
    wqk_t = nc.dram_tensor("wqk_t", [H, QKR], BF16, kind="ExternalInput").ap()
    wv_t = nc.dram_tensor("wv_t", [H, DG], BF16, kind="ExternalInput").ap()
    wo_t = nc.dram_tensor("wo_t", [DG, H], BF16, kind="ExternalInput").ap()
    bqk = nc.dram_tensor("bqk", [QKR], FP32, kind="ExternalInput").ap()
    mask = nc.dram_tensor("mask", [S], FP32, kind="ExternalInput").ap()
    out_t = nc.dram_tensor("out_t", [H, S], BF16, kind="ExternalOutput").ap()

    with tile.TileContext(nc) as tc:
        _body(tc, x_t, wqk_t, wv_t, wo_t, bqk, mask, out_t)
    nc.compile()
    return nc


def _get_nc():
    global _NC_CACHE
    if _NC_CACHE is None:
        _NC_CACHE = _build()
    return _NC_CACHE


def make_in_maps(hidden_states, attention_mask, w_qkv, b_qkv, w_out):
    import ml_dtypes

    bf16 = ml_dtypes.bfloat16
    in_maps = []
    for core in range(NCORES):
        b, g = divmod(core, NGROUP)
        wq = w_qkv[0 * H + g * DG:0 * H + (g + 1) * DG]
        wk = w_qkv[1 * H + g * DG:1 * H + (g + 1) * DG]
        wv = w_qkv[2 * H + g * DG:2 * H + (g + 1) * DG]
        in_maps.append({
            "x_t": np.ascontiguousarray(hidden_states[b].T).astype(bf16),
            "wqk_t": np.ascontiguousarray(
                np.concatenate([wq, wk], 0).T).astype(bf16),
            "wv_t": np.ascontiguousarray(wv.T).astype(bf16),
            "wo_t": np.ascontiguousarray(
                w_out[:, g * DG:(g + 1) * DG].T).astype(bf16),
            "bqk": np.ascontiguousarray(
                np.concatenate([b_qkv[g * DG:(g + 1) * DG],
                                b_qkv[H + g * DG:H + (g + 1) * DG]])),
            "mask": np.ascontiguousarray(attention_mask[b]),
        })
    return in_maps


def kernel(hidden_states, attention_mask, w_qkv, b_qkv, w_out, b_out):
    global LAST_RESULT
    hidden_states = np.asarray(hidden_states, dtype=np.float32)
    attention_mask = np.asarray(attention_mask, dtype=np.float32)
    w_qkv = np.asarray(w_qkv, dtype=np.float32)
    b_qkv = np.asarray(b_qkv, dtype=np.float32)
    w_out = np.asarray(w_out, dtype=np.float32)
    b_out = np.asarray(b_out, dtype=np.float32)

    nc = _get_nc()
    in_maps = make_in_maps(hidden_states, attention_mask, w_qkv, b_qkv, w_out)

    import os
    trace = bool(int(os.environ.get("KERNEL_TRACE", "0")))
    res = run_bass_kernel_spmd(
        nc, in_maps, core_ids=list(range(NCORES)), trace=trace,
    )
    LAST_RESULT = res

    out = np.zeros((B, S, H), np.float32)
    vbias = w_out @ b_qkv[2 * H:]          # exact v-bias correction
    for b in range(B):
        acc = res.results[b * NGROUP + 0]["out_t"].astype(np.float32)
        for g in range(1, NGROUP):
            acc = acc + res.results[b * NGROUP + g]["out_t"].astype(np.float32)
        out[b] = acc.T + b_out + vbias
    return out


# revision 29
# speedup vs baseline: 1.0030x; 1.0030x over previous
"""Multi-head attention (B=2, S=2048, H=1024, 16 heads) on 8 TRN2 NeuronCores.

Sharding: tensor-parallel over heads x data-parallel over batch.
core = b * 4 + g handles batch b and head-group g (4 heads, 256 channels).

Device-side dataflow (bf16 operands, fp32 PSUM accumulation):
  - Everything stays in "transposed space" so every matmul contracts over the
    partition dim with no on-device transposes:
      x_t    [H, S]      = hidden[b].T                      (host-transposed)
      qk_T   [512, S]    = (Wqk_g x_t)                      rows: q(4 heads), k(4 heads)
      v      [S, 256]    = x w_v.T  (natural layout; lhsT = x_t chunks)
      st     [128k, q]   = k_T_h^T-contracted scores (transposed scores)
      pt     = exp(st * scale + mask[k])                    (ACT, bias = per-partition mask)
      av     [128, q]    = v_aug^T pt ; rows 0:64 = unnormalized out.T,
                           rows 64:128 = Z[q] replicated (v_aug cols 64:128 == 1)
      attn_T [256, S]    = av[:64] * reciprocal(av[64:128])
      out_t  [H, S]      = Wo_g^T-contracted partial output (transposed)
  - Host sums the 4 group partials per batch, transposes back, and adds the
    exact bias corrections: b_out plus w_out @ b_v (the ones-augmented-V
    identity makes the v-bias a constant channel offset).
"""

import numpy as np

import concourse.tile as tile
from concourse import bacc, mybir
from concourse.bass_utils import run_bass_kernel_spmd

B, S, H = 2, 2048, 1024
NH, HD = 16, 64
NCORES = 8
NGROUP = 4              # head groups = cores per batch
HPG = NH // NGROUP      # 4 heads per group
DG = HPG * HD           # 256 channels per group
P = 128
SCALE = float(HD) ** -0.5

FP32 = mybir.dt.float32
FP32R = mybir.dt.float32r
BF16 = mybir.dt.bfloat16

S_TILES = S // P        # 16 key/token tiles
HC = H // P             # 8 contraction chunks over H
QKR = 2 * DG            # 512 q+k rows
QKC = QKR // P          # 4 chunks of qk rows
TQ = 512                # token quarter for qkv streaming
NQT = S // TQ           # 4
QT = 1024               # q tile width in attention / out-proj
NQ = S // QT            # 2

_NC_CACHE = None
LAST_RESULT = None      # BassKernelResults of the most recent run (for test.py)


def _body(tc, x_t, wqk_t, wv_t, wo_t, bqk, mask, out_t):
    nc = tc.nc
    with (
        tc.tile_pool(name="const", bufs=1) as const,
        tc.tile_pool(name="big", bufs=1) as big,
        tc.tile_pool(name="pt_pool", bufs=8) as pt_pool,
        tc.tile_pool(name="rz_pool", bufs=2) as rz_pool,
        tc.tile_pool(name="osb_pool", bufs=3) as osb_pool,
        tc.tile_pool(name="ps", bufs=2, space="PSUM") as ps,
        tc.tile_pool(name="avps", bufs=2, space="PSUM") as avps,
        tc.tile_pool(name="iops", bufs=2, space="PSUM") as iops,
    ):
        # ---------- input DMAs, ordered by when compute needs them ----------
        # x chunks first (gates everything), split over three issuing engines
        # so descriptor generation (~0.6us per dma_start per sequencer)
        # overlaps; w_out / mask / biases land last (needed latest).
        x_sb = big.tile([P, HC, S], BF16, name="x_sb")
        x_r = x_t.rearrange("(c p) s -> p c s", p=P)
        dma_engines = (nc.sync, nc.scalar, nc.gpsimd)
        for hc in range(HC):
            dma_engines[hc % 3].dma_start(x_sb[:, hc, :], x_r[:, hc, :])

        wqk_sb = const.tile([P, HC, QKR], BF16, name="wqk_sb")
        nc.sync.dma_start(wqk_sb[:], wqk_t.rearrange("(c p) r -> p c r", p=P))
        wv_sb = const.tile([P, HC, DG], BF16, name="wv_sb")
        nc.scalar.dma_start(wv_sb[:], wv_t.rearrange("(c p) r -> p c r", p=P))
        bqk_sb = const.tile([P, QKC], FP32, name="bqk_sb")
        nc.scalar.dma_start(bqk_sb[:], bqk.rearrange("(c p) -> p c", p=P))
        mask_sb = const.tile([P, S_TILES], FP32, name="mask_sb")
        nc.scalar.dma_start(mask_sb[:], mask.rearrange("(c p) -> p c", p=P))
        wo_sb = const.tile([P, DG // P, H], BF16, name="wo_sb")
        nc.gpsimd.dma_start(wo_sb[:], wo_t.rearrange("(c p) r -> p c r", p=P))

        qk_sb = big.tile([P, QKC, S], BF16, name="qk_sb")
        # v_aug: per token tile / head: [v (64 cols) | ones (64 cols)]
        v_sb = big.tile([P, S_TILES, HPG, 2 * HD], BF16, name="v_sb")
        attn_sb = big.tile([P, DG // P, S], BF16, name="attn_sb")

        # ones half of v_aug: memset a bf16 staging tile, copy per token tile
        ones_sb = const.tile([P, HPG, HD], BF16, name="ones_sb")
        nc.vector.memset(ones_sb[:], 1.0)
        for tt in range(S_TILES):
            nc.vector.tensor_copy(v_sb[:, tt, :, HD:2 * HD], ones_sb[:])

        # ---------- qkv projection ----------
        # One [128,512] psum slot per group, double-buffered, v and qk groups
        # interleaved for PE density.  qk chunk order (0, 2, 1, 3): q+k of
        # head pair 0 first so attention starts while pair 1 still projects.
        def v_group(tp):
            # two token tiles (2*tp, 2*tp+1) side by side in one psum slot
            v_ps = iops.tile([P, 512], FP32, name="v_ps", tag="io")
            for half in range(2):
                tt = 2 * tp + half
                for hc in range(HC):
                    nc.tensor.matmul(
                        v_ps[:, half * DG:(half + 1) * DG],
                        lhsT=x_sb[:, hc, tt * P:(tt + 1) * P],
                        rhs=wv_sb[:, hc, :],
                        start=(hc == 0),
                        stop=(hc == HC - 1),
                    )
            nc.vector.tensor_copy(
                v_sb[:, 2 * tp:2 * tp + 2, :, 0:HD],
                v_ps[:].rearrange("p (t h d) -> p t h d", t=2, d=HD),
            )

        def qk_group(rc, i):
            qk_ps = iops.tile([P, 512], FP32, name="qk_ps", tag="io")
            for hc in range(HC):
                nc.tensor.matmul(
                    qk_ps[:],
                    lhsT=wqk_sb[:, hc, rc * P:(rc + 1) * P],
                    rhs=x_sb[:, hc, i * 512:(i + 1) * 512],
                    start=(hc == 0),
                    stop=(hc == HC - 1),
                )
            nc.vector.tensor_scalar_add(
                qk_sb[:, rc, i * 512:(i + 1) * 512],
                qk_ps[:],
                bqk_sb[:, rc:rc + 1],
            )

        # Window-0 dependencies first: q cols 0-511, then k/v in token order so
        # attention window 0 starts consuming k-tile kt / v-tile tt as they
        # appear; remaining q windows follow.
        qk_group(0, 0)
        for j in range(4):
            qk_group(2, j)
            v_group(2 * j)
            v_group(2 * j + 1)
            if j > 0:
                qk_group(0, j)

        # ---------- attention: head PAIRS packed into PE row-groups ----------
        # Heads (2*qc, 2*qc+1) live at partition offsets 0/64 of qk chunk qc,
        # so their score matmuls land in disjoint row groups (tile_position
        # (0,0) / (64,0)) and execute concurrently.  Their 512-wide score
        # tiles sit side by side in one [128,1024] PSUM tile so a single
        # N=1024 exp covers both (mask bias depends only on the k-partition).
        o_r = out_t.rearrange("(c p) s -> p c s", p=P)

        def out_proj_window(q5):
            """Output projection for one 512-wide q window (both attn chunks
            must be complete there).  Two j-chunks packed per psum slot."""
            qlo = q5 * 512
            for j in range(H // P):
                o_ps = iops.tile([P, 512], FP32, name="o_ps", tag="io")
                for kc in range(DG // P):
                    nc.tensor.matmul(
                        o_ps[:],
                        lhsT=wo_sb[:, kc, j * P:(j + 1) * P],
                        rhs=attn_sb[:, kc, qlo:qlo + 512],
                        start=(kc == 0),
                        stop=(kc == DG // P - 1),
                    )
                o_sb = osb_pool.tile([P, 512], BF16, name="o_sb", tag="osb")
                nc.vector.tensor_copy(o_sb[:], o_ps[:])
                nc.sync.dma_start(o_r[:, j, qlo:qlo + 512], o_sb[:])

        for qc in range(HPG // 2):
            for q5 in range(S // 512):
                qlo = q5 * 512
                av0 = avps.tile([P, 512], FP32, name="av0", tag="av")
                av1 = avps.tile([P, 512], FP32, name="av1", tag="av")
                for kt in range(S_TILES):
                    st = ps.tile([P, QT], FP32, name="st", tag="mm")
                    for half in range(2):
                        off = half * HD
                        nc.tensor.matmul(
                            st[:, half * 512:(half + 1) * 512],
                            lhsT=qk_sb[off:off + HD, 2 + qc,
                                       kt * P:(kt + 1) * P],
                            rhs=qk_sb[off:off + HD, qc, qlo:qlo + 512],
                            start=True,
                            stop=True,
                        )
                    pt = pt_pool.tile([P, QT], BF16, name="pt", tag="pt")
                    nc.scalar.activation(
                        pt[:], st[:],
                        mybir.ActivationFunctionType.Exp,
                        bias=mask_sb[:, kt:kt + 1],
                        scale=SCALE,
                    )
                    for half, av in ((0, av0), (1, av1)):
                        nc.tensor.matmul(
                            av[:],
                            lhsT=v_sb[:, kt, 2 * qc + half, :],
                            rhs=pt[:, half * 512:(half + 1) * 512],
                            start=(kt == 0),
                            stop=(kt == S_TILES - 1),
                        )
                for half, av in ((0, av0), (1, av1)):
                    off = half * HD
                    zc = rz_pool.tile([HD, 512], FP32, name="zc", tag="zc")
                    nc.vector.tensor_copy(zc[:], av[HD:2 * HD, :])
                    rz = rz_pool.tile([HD, 512], FP32, name="rz", tag="rz")
                    nc.vector.reciprocal_approx_fast(rz[:], zc[:])
                    nc.vector.tensor_mul(
                        attn_sb[off:off + HD, qc, qlo:qlo + 512],
                        av[0:HD, :],
                        rz[:],
                    )
                if qc == 0:
                    # pair 1's projection, emitted AFTER this window's
                    # attention ops: lower scheduler priority, so it fills PE
                    # gaps while ACT stays on the exp critical path
                    qk_group(1, q5)
                    qk_group(3, q5)
                else:
                    out_proj_window(q5)


def _build():
    nc = bacc.Bacc(
        "TRN2",
        target_bir_lowering=False,
        debug=False,
        enable_asserts=True,
        num_devices=NCORES,
    )
    x_t = nc.dram_tensor("x_t", [H, S], BF16, kind="ExternalInput").ap()
    wqk_t = nc.dram_tensor("wqk_t", [H, QKR], BF16, kind="ExternalInput").ap()
    wv_t = nc.dram_tensor("wv_t", [H, DG], BF16, kind="ExternalInput").ap()
    wo_t = nc.dram_tensor("wo_t", [DG, H], BF16, kind="ExternalInput").ap()
    bqk = nc.dram_tensor("bqk", [QKR], FP32, kind="ExternalInput").ap()
    mask = nc.dram_tensor("mask", [S], FP32, kind="ExternalInput").ap()
    out_t = nc.dram_tensor("out_t", [H, S], BF16, kind="ExternalOutput").ap()

    with tile.TileContext(nc) as tc:
        _body(tc, x_t, wqk_t, wv_t, wo_t, bqk, mask, out_t)
    nc.compile()
    return nc


def _get_nc():
    global _NC_CACHE
    if _NC_CACHE is None:
        _NC_CACHE = _build()
    return _NC_CACHE


def make_in_maps(hidden_states, attention_mask, w_qkv, b_qkv, w_out):
    import ml_dtypes

    bf16 = ml_dtypes.bfloat16
    in_maps = []
    for core in range(NCORES):
        b, g = divmod(core, NGROUP)
        wq = w_qkv[0 * H + g * DG:0 * H + (g + 1) * DG]
        wk = w_qkv[1 * H + g * DG:1 * H + (g + 1) * DG]
        wv = w_qkv[2 * H + g * DG:2 * H + (g + 1) * DG]
        in_maps.append({
            "x_t": np.ascontiguousarray(hidden_states[b].T).astype(bf16),
            "wqk_t": np.ascontiguousarray(
                np.concatenate([wq, wk], 0).T).astype(bf16),
            "wv_t": np.ascontiguousarray(wv.T).astype(bf16),
            "wo_t": np.ascontiguousarray(
                w_out[:, g * DG:(g + 1) * DG].T).astype(bf16),
            "bqk": np.ascontiguousarray(
                np.concatenate([b_qkv[g * DG:(g + 1) * DG],
                                b_qkv[H + g * DG:H + (g + 1) * DG]])),
            "mask": np.ascontiguousarray(attention_mask[b]),
        })
    return in_maps


def kernel(hidden_states, attention_mask, w_qkv, b_qkv, w_out, b_out):
    global LAST_RESULT
    hidden_states = np.asarray(hidden_states, dtype=np.float32)
    attention_mask = np.asarray(attention_mask, dtype=np.float32)
    w_qkv = np.asarray(w_qkv, dtype=np.float32)
    b_qkv = np.asarray(b_qkv, dtype=np.float32)
    w_out = np.asarray(w_out, dtype=np.float32)
    b_out = np.asarray(b_out, dtype=np.float32)

    nc = _get_nc()
    in_maps = make_in_maps(hidden_states, attention_mask, w_qkv, b_qkv, w_out)

    import os
    trace = bool(int(os.environ.get("KERNEL_TRACE", "0")))
    res = run_bass_kernel_spmd(
        nc, in_maps, core_ids=list(range(NCORES)), trace=trace,
    )
    LAST_RESULT = res

    out = np.zeros((B, S, H), np.float32)
    vbias = w_out @ b_qkv[2 * H:]          # exact v-bias correction
    for b in range(B):
        acc = res.results[b * NGROUP + 0]["out_t"].astype(np.float32)
        for g in range(1, NGROUP):
            acc = acc + res.results[b * NGROUP + g]["out_t"].astype(np.float32)
        out[b] = acc.T + b_out + vbias
    return out
